# revision 24
# baseline (speedup 1.0000x reference)
"""Decode-style single-query attention (B=32, N=8192, D=256, H=8) on 8 TRN2 cores.

Strategy: pure data-parallel over batch (4 batches/core, no collectives).
Per batch, the single query makes K/V projections unnecessary:
  scores[n,h] = X[n,:] @ kq[:,h],  kq = Wk-head-blocks @ (q@Wq + bq)  (bk cancels in softmax)
  pooled[h,:] = softmax(scores*scale)[h,:] @ X[n,:]   (flash-style, one pass over X)
  attn[e]    = pooled[e//32,:] @ Wv[:,e] + bv[e]
  out        = q_raw + attn @ Wo + bo

X streams through SBUF once (f32 load on SWDGE, DVE cast to bf16). The scores
matmul contracts over d, so a transposed copy of each slab is made with the
xbar DMA-transpose engine (SP HWDGE ring); scores run kq-stationary with wide
512-col moving operands; exp on ACT emits the softmax denominator for free via
accum_out; p^T (the pooling matmul contracts over n) comes from a second small
xbar transpose on the ACT HWDGE ring. The pooling matmuls accumulate each
batch in its own PSUM bank, one slab behind the scores pipeline.

The 4 batches are interleaved slab-by-slab: four independent dependency
chains in flight hide the ~7-hop cross-engine latency per slab and keep the
TensorEngine warm.
"""

import os
import sys

sys.path.insert(0, "/opt/trn_rl_repo")

from contextlib import ExitStack

import ml_dtypes
import numpy as np

import concourse.bass as bass
import concourse.tile as tile
from concourse import bacc, mybir
from concourse.bass_utils import run_bass_kernel_spmd

F32 = mybir.dt.float32
BF16 = mybir.dt.bfloat16
ts = bass.ts

B, D, H = 32, 256, 8
HP = 16  # head dim padded to 16 partitions (rows 8:16 are zero-scores)
N = int(os.environ.get("K_N", "8192"))
DH = D // H
NCORES = 8
BL = B // NCORES  # batches per core
SCALE = 1.0 / float(np.sqrt(DH))

SLAB = int(os.environ.get("K_SLAB", "1024"))  # rows of X per streamed slab
NSUB = SLAB // 128  # 128-row subtiles per slab
NHALF = NSUB // 4  # 512-col score matmul groups per slab
NSLAB = N // SLAB  # slabs per batch
XT_MODE = os.environ.get("K_XT_MODE", "xbar")  # 'xbar' | 'pe'
CAST_MODE = os.environ.get("K_CAST", "dma")  # 'dve' | 'dma' (SWDGE cast)

EXP = mybir.ActivationFunctionType.Exp

_cache = {}


def build_graph(reps=1):
    nc = bacc.Bacc("TRN2", target_bir_lowering=False, debug=False, num_devices=NCORES)

    x_ext = nc.declare_dram_parameter("x", [BL, N, D], F32, isOutput=False)
    wq_ext = nc.declare_dram_parameter("Wq", [D, D], F32, isOutput=False)
    wkT_ext = nc.declare_dram_parameter("WkT", [D, D], F32, isOutput=False)
    wv_ext = nc.declare_dram_parameter("Wv", [D, D], F32, isOutput=False)
    wo_ext = nc.declare_dram_parameter("Wo", [D, D], F32, isOutput=False)
    bqc_ext = nc.declare_dram_parameter("bqc", [128, 2], F32, isOutput=False)
    bvc_ext = nc.declare_dram_parameter("bvc", [128, 2], F32, isOutput=False)
    bo_ext = nc.declare_dram_parameter("bo", [1, D], F32, isOutput=False)
    mqc_ext = nc.declare_dram_parameter("mqc", [128, 2, HP], F32, isOutput=False)
    mh_ext = nc.declare_dram_parameter("maskh", [H, D], F32, isOutput=False)
    ones16_ext = nc.declare_dram_parameter("ones16", [128, 1], BF16, isOutput=False)
    id32_ext = nc.declare_dram_parameter("ident32", [128, 128], F32, isOutput=False)
    id16_ext = nc.declare_dram_parameter("ident16", [128, 128], BF16, isOutput=False)
    out_ext = nc.declare_dram_parameter("out", [BL, D], F32, isOutput=True)

    with tile.TileContext(nc) as tc, ExitStack() as ctx:
        const = ctx.enter_context(tc.tile_pool(name="const", bufs=1))
        stage = ctx.enter_context(tc.tile_pool(name="stage", bufs=1))
        xfp = ctx.enter_context(tc.tile_pool(name="xf", bufs=8))
        xbp = ctx.enter_context(tc.tile_pool(name="xb", bufs=12))
        xtp = ctx.enter_context(tc.tile_pool(name="xt", bufs=8))
        psp = ctx.enter_context(tc.tile_pool(name="pst", bufs=8))
        ptp = ctx.enter_context(tc.tile_pool(name="pt", bufs=10))
        ep = ctx.enter_context(tc.tile_pool(name="ep", bufs=2))
        bpool = ctx.enter_context(tc.tile_pool(name="bp", bufs=1))
        sps = ctx.enter_context(tc.tile_pool(name="sps", bufs=3, space="PSUM"))
        accp = ctx.enter_context(tc.tile_pool(name="accp", bufs=1, space="PSUM"))
        epsum = ctx.enter_context(tc.tile_pool(name="epsum", bufs=1, space="PSUM"))
        if XT_MODE == "pe":
            xtps = ctx.enter_context(tc.tile_pool(name="xtps", bufs=2, space="PSUM"))

        # ---- constants ----
        ld = nc.scalar  # ACT HWDGE ring for small/constant loads

        wq_st = stage.tile([128, 2, D], F32, tag="stage")
        ld.dma_start(wq_st[:], wq_ext.ap().rearrange("(c p) e -> p c e", p=128))
        wq16 = const.tile([128, 2, D], BF16)  # Wq[d,e] d-chunked
        nc.vector.tensor_copy(wq16[:], wq_st[:])

        wkT_st = stage.tile([128, 2, D], F32, tag="stage")
        ld.dma_start(wkT_st[:], wkT_ext.ap().rearrange("(c p) d -> p c d", p=128))
        wkT16 = const.tile([128, 2, D], BF16)  # WkT[e,d] e-chunked
        nc.vector.tensor_copy(wkT16[:], wkT_st[:])

        wv_st = stage.tile([128, 2, D], F32, tag="stage")
        ld.dma_start(wv_st[:], wv_ext.ap().rearrange("(c p) e -> p c e", p=128))
        wv16 = const.tile([128, 2, D], BF16)  # Wv[d,e] d-chunked
        nc.vector.tensor_copy(wv16[:], wv_st[:])

        wo_st = stage.tile([128, 2, D], F32, tag="stage")
        ld.dma_start(wo_st[:], wo_ext.ap().rearrange("(c p) e -> p c e", p=128))
        wo16 = const.tile([128, 2, D], BF16)  # Wo[e,e'] e-chunked
        nc.vector.tensor_copy(wo16[:], wo_st[:])

        bqc_sb = const.tile([128, 2], F32)
        ld.dma_start(bqc_sb[:], bqc_ext.ap())
        bvc_sb = const.tile([128, 2], F32)
        ld.dma_start(bvc_sb[:], bvc_ext.ap())
        bo_sb = const.tile([1, D], F32)
        ld.dma_start(bo_sb[:], bo_ext.ap())
        mqc_sb = const.tile([128, 2, HP], F32)
        ld.dma_start(mqc_sb[:], mqc_ext.ap())
        mh_sb = const.tile([H, D], F32)
        ld.dma_start(mh_sb[:], mh_ext.ap())
        ones16_sb = const.tile([128, 1], BF16)
        ld.dma_start(ones16_sb[:], ones16_ext.ap())
        id32_sb = const.tile([128, 128], F32)
        ld.dma_start(id32_sb[:], id32_ext.ap())
        id16_sb = const.tile([128, 128], BF16)
        ld.dma_start(id16_sb[:], id16_ext.ap())

        def prologue(b, st):
            """q, kq (kq zero-padded to HP cols) for batch b."""
            qT = ep.tile([128, 2], F32, tag="qT")
            ld.dma_start(qT[:], x_ext.ap()[b, 0, :].rearrange("(c p) -> p c", p=128))
            qT16 = ep.tile([128, 2], BF16, tag="qT16")
            nc.vector.tensor_copy(qT16[:], qT[:])
            qn = ep.tile([1, D], F32, tag="qn")
            ld.dma_start(qn[:], x_ext.ap()[b, 0:1, :])
            st["qbo"] = bpool.tile([1, D], F32, tag=f"qbo{b}", name=f"qbo{b}")
            nc.vector.tensor_add(st["qbo"][:], qn[:], bo_sb[:])

            qf_ps = epsum.tile([128, 2], F32, tag="eps")
            for mc in range(2):
                for kc in range(2):
                    nc.tensor.matmul(
                        qf_ps[:, mc : mc + 1],
                        wq16[:, kc, ts(mc, 128)],
                        qT16[:, kc : kc + 1],
                        start=(kc == 0),
                        stop=(kc == 1),
                    )
            qfb = ep.tile([128, 2], F32, tag="qfb")
            nc.vector.tensor_add(qfb[:], qf_ps[:], bqc_sb[:])

            sq16 = ep.tile([128, 2, HP], BF16, tag="sq16")
            for c in range(2):
                nc.vector.tensor_scalar_mul(sq16[:, c, :], mqc_sb[:, c, :], qfb[:, c : c + 1])

            kqT_ps = epsum.tile([HP, D], F32, tag="eps")
            for c in range(2):
                nc.tensor.matmul(
                    kqT_ps[:], sq16[:, c, :], wkT16[:, c, :], start=(c == 0), stop=(c == 1)
                )
            kqT_sb = ep.tile([HP, D], F32, tag="kqT")
            nc.vector.tensor_copy(kqT_sb[:], kqT_ps[:])

            kq_ps = epsum.tile([128, 2, HP], F32, tag="eps")
            for c in range(2):
                nc.tensor.transpose(kq_ps[:, c, :], kqT_sb[:, ts(c, 128)], id32_sb[:HP, :HP])
            st["kq16"] = bpool.tile([128, 2, HP], BF16, tag=f"kq16_{b}", name=f"kq16_{b}")
            for c in range(2):
                nc.vector.tensor_copy(st["kq16"][:, c, :], kq_ps[:, c, :])

            st["acc"] = accp.tile([H, D], F32, tag=f"acc{b}", name=f"acc{b}")
            st["lparts"] = bpool.tile([HP, NSLAB * NHALF], F32, tag=f"lp{b}", name=f"lp{b}")
            st["prev"] = None

        def stream_slab(b, s, st):
            xb = xbp.tile([128, NSUB, D], BF16, tag="xb")
            src = x_ext.ap()[b, s * SLAB : (s + 1) * SLAB, :].rearrange(
                "(p j) d -> p j d", p=128
            )
            if CAST_MODE == "dma":
                nc.gpsimd.dma_start(xb[:], src)  # f32 -> bf16 cast in DMA
            else:
                xf = xfp.tile([128, NSUB, D], F32, tag="xf")
                nc.gpsimd.dma_start(xf[:], src)
                nc.vector.tensor_copy(xb[:], xf[:])

            # xt[:, t*2+c, :] = X[t*128:(t+1)*128, c*128:(c+1)*128].T
            xt = xtp.tile([128, 2 * NSUB, 128], BF16, tag="xt")
            if XT_MODE == "xbar":
                nc.sync.dma_start(
                    out=xt[:], in_=xb[:].rearrange("p t d -> p (t d)"), transpose=True
                )
            else:
                for half in range(NSUB // 2):
                    tp = xtps.tile([128, 512], BF16, tag="xtps")
                    for j in range(4):
                        t = half * 2 + j // 2
                        c = j % 2
                        nc.tensor.transpose(
                            tp[:, ts(j, 128)], xb[:, t, ts(c, 128)], id16_sb[:]
                        )
                    if half % 2 == 0:
                        nc.vector.tensor_copy(xt[:, ts(half, 4), :], tp[:])
                    else:
                        nc.scalar.copy(xt[:, ts(half, 4), :], tp[:])

            # wide scores + exp per 512-row group
            pstr = psp.tile([HP, NSUB * 128], BF16, tag="pstr")
            xtv = xt[:].rearrange("p (t c) n -> p c t n", c=2)
            kq16 = st["kq16"]
            for hf in range(NHALF):
                s_ps = sps.tile([HP, 512], F32, tag="s")
                for c in range(2):
                    nc.tensor.matmul(
                        s_ps[:],
                        kq16[:, c, :],
                        xtv[:, c, hf * 4 : (hf + 1) * 4, :],
                        start=(c == 0),
                        stop=(c == 1),
                    )
                nc.scalar.activation(
                    pstr[:, hf * 512 : (hf + 1) * 512],
                    s_ps[:],
                    EXP,
                    scale=SCALE,
                    accum_out=st["lparts"][:, s * NHALF + hf : s * NHALF + hf + 1],
                )

            # p^T for the pooling matmuls via xbar transpose (ACT ring)
            pt = ptp.tile([128, NSUB, HP], BF16, tag="pt")
            nc.scalar.dma_start(out=pt[:], in_=pstr[:], transpose=True)

            if st["prev"] is not None:
                emit_pooled(st, first=(s == 1), last=False)
            st["prev"] = (pt, xb)

        def emit_pooled(st, first, last):
            pt_prev, xb_prev = st["prev"]
            for t in range(NSUB):
                nc.tensor.matmul(
                    st["acc"][:],
                    pt_prev[:, t, 0:H],
                    xb_prev[:, t, :],
                    start=(first and t == 0),
                    stop=(last and t == NSUB - 1),
                )

        def epilogue(b, st):
            emit_pooled(st, first=(NSLAB == 1), last=True)
            lsum = ep.tile([HP, 1], F32, tag="lsum")
            nc.vector.tensor_reduce(
                lsum[:], st["lparts"][:], axis=mybir.AxisListType.X, op=mybir.AluOpType.add
            )
            linv = ep.tile([H, 1], F32, tag="linv")
            nc.vector.reciprocal(linv[:], lsum[0:H, :])
            pooled16 = ep.tile([H, D], BF16, tag="pooled")
            nc.vector.tensor_scalar_mul(pooled16[:], st["acc"][:], linv[:, 0:1])

            pt_ps = epsum.tile([128, 2, H], BF16, tag="eps")
            for c in range(2):
                nc.tensor.transpose(pt_ps[:, c, :], pooled16[:, ts(c, 128)], id16_sb[:H, :H])
            pt16 = ep.tile([128, 2, H], BF16, tag="pt16")
            for c in range(2):
                nc.vector.tensor_copy(pt16[:, c, :], pt_ps[:, c, :])

            y_ps = epsum.tile([H, D], F32, tag="eps")
            for c in range(2):
                nc.tensor.matmul(
                    y_ps[:], pt16[:, c, :], wv16[:, c, :], start=(c == 0), stop=(c == 1)
                )
            ym16 = ep.tile([H, D], BF16, tag="ym")
            nc.vector.tensor_mul(ym16[:], y_ps[:], mh_sb[:])

            attn_ps = epsum.tile([1, D], F32, tag="eps")
            nc.tensor.matmul(attn_ps[:], ones16_sb[:H, 0:1], ym16[:], start=True, stop=True)
            attn_sb = ep.tile([1, D], F32, tag="attn")
            nc.vector.tensor_copy(attn_sb[:], attn_ps[:])

            at_ps = epsum.tile([128, 2], F32, tag="eps")
            for c in range(2):
                nc.tensor.transpose(
                    at_ps[:, c : c + 1], attn_sb[:, ts(c, 128)], id32_sb[:1, :1]
                )
            at16 = ep.tile([128, 2], BF16, tag="at16")
            for c in range(2):
                nc.vector.tensor_add(
                    at16[:, c : c + 1], at_ps[:, c : c + 1], bvc_sb[:, c : c + 1]
                )

            res_ps = epsum.tile([1, D], F32, tag="eps")
            for c in range(2):
                nc.tensor.matmul(
                    res_ps[:], at16[:, c : c + 1], wo16[:, c, :], start=(c == 0), stop=(c == 1)
                )
            out_sb = ep.tile([1, D], F32, tag="out")
            nc.vector.tensor_add(out_sb[:], res_ps[:], st["qbo"][:])
            nc.sync.dma_start(out_ext.ap()[b : b + 1, :], out_sb[:])

        # ---- interleave the BL batches slab-by-slab ----
        for _ in range(reps):
            states = [dict() for _ in range(BL)]
            for b in range(BL):
                prologue(b, states[b])
            for s in range(NSLAB):
                for b in range(BL):
                    stream_slab(b, s, states[b])
            for b in range(BL):
                epilogue(b, states[b])

    nc.compile()
    return nc


def _host_consts():
    e = np.arange(D)
    mq = (e[:, None] // DH == np.arange(HP)[None, :]).astype(np.float32)  # [D, HP]
    consts = {
        "mqc": np.ascontiguousarray(mq.reshape(2, 128, HP).transpose(1, 0, 2)),
        "maskh": np.ascontiguousarray(
            (np.arange(H)[:, None] == e[None, :] // DH).astype(np.float32)
        ),
        "ones16": np.ones((128, 1), ml_dtypes.bfloat16),
        "ident32": np.eye(128, dtype=np.float32),
        "ident16": np.eye(128, dtype=ml_dtypes.bfloat16),
    }
    return consts


def kernel(**inputs):
    x = np.ascontiguousarray(np.asarray(inputs["x"], dtype=np.float32))
    Wq = np.ascontiguousarray(np.asarray(inputs["Wq"], dtype=np.float32))
    bq = np.asarray(inputs["bq"], dtype=np.float32)
    Wk = np.ascontiguousarray(np.asarray(inputs["Wk"], dtype=np.float32))
    Wv = np.ascontiguousarray(np.asarray(inputs["Wv"], dtype=np.float32))
    Wo = np.ascontiguousarray(np.asarray(inputs["Wo"], dtype=np.float32))
    bv = np.asarray(inputs["bv"], dtype=np.float32)
    bo = np.asarray(inputs["bo"], dtype=np.float32)
    # bk is unused: softmax is shift-invariant and Q.bk is constant over keys.

    if "nc" not in _cache:
        _cache["nc"] = build_graph()
    nc = _cache["nc"]

    consts = _host_consts()
    shared = {
        "Wq": Wq,
        "WkT": np.ascontiguousarray(Wk.T),
        "Wv": Wv,
        "Wo": Wo,
        "bqc": np.ascontiguousarray(bq.reshape(2, 128).T),
        "bvc": np.ascontiguousarray(bv.reshape(2, 128).T),
        "bo": np.ascontiguousarray(bo.reshape(1, D)),
        **consts,
    }
    in_maps = []
    for c in range(NCORES):
        m = dict(shared)
        m["x"] = np.ascontiguousarray(x[c * BL : (c + 1) * BL])
        in_maps.append(m)

    trace = bool(int(os.environ.get("K_TRACE", "0")))
    if trace:
        try:
            import axon_prof

            axon_prof.install()
        except Exception as e:
            print(f"axon_prof install failed: {e}")
    res = run_bass_kernel_spmd(
        nc,
        in_maps,
        core_ids=list(range(NCORES)),
        trace=trace,
        tmpdir=os.environ.get("K_TRACE_DIR") or None,
    )
    _cache["last_results"] = res
    out = np.concatenate([res.results[i]["out"] for i in range(NCORES)], axis=0)
    return out.reshape(B, 1, D).astype(np.float32)
